# revision 8
# baseline (speedup 1.0000x reference)
"""Trainium2 Bass kernel: MeshLaplacianSmoothing loss (uniform Laplacian).

Computes  sum_{n,v} || nbr(v)/deg(v) - x_v ||_2 / (V*N)  over N meshes.

The harness topology is a triangulated regular G x G grid (G=1000), so the
edge gather/scatter reduces to a fixed 6-neighbor stencil:
    neighbors of (i,j): (i,j-1) (i,j+1) (i-1,j) (i+1,j) (i-1,j-1) (i+1,j+1)
kernel() verifies this against the provided edge list at runtime (exact
comparison) and falls back to a host computation for any other topology.

Device strategy (8 NeuronCores, SPMD, grid rows sharded 125/core):
  - One [128, 3008] fp8(e4m3) PLANAR slab per (core, mesh): three coord
    planes of 1002 cols (j = -1..1000, zero-padded ends) at offsets
    1002*d; grid rows r-1..r+126 on partitions (halo zero-padded).
  - Stencil on the PE, 2 passes per 500-col half-plane into PSUM f32:
    pass A fp8 DoubleRow with an overlapping rhs AP [k][ko: stride 2]
    [n: stride 1] applying the j-1 band (ul,l) and j+1 band (r,dr) in
    one pass; pass B plain fp8 applies the center band {up,-deg_mid,
    down}. PSUM: one 2-bank tile per (mesh, plane), 4 rotating tags.
  - Squares: x-plane squared PSUM->SBUF bf16 (ACT Square or a custom
    DVE op, split per mesh to balance engines); y and z planes via a
    runtime-registered custom DVE op SQ_ACC_ANT: out = sq(psum) + sbuf
    (fused square+accumulate, ~0.66ns/col measured) -> acc bf16.
  - One batched ACT Sqrt over all 4 meshes' acc [128, 4x1000] with
    per-row scale w_mid^2 and accum_out -> [128,1] partial per core.
  - j-boundary columns (j=0,999; deg != deg_mid there) are NOT fixed on
    device: the host subtracts the (deterministically emulated) device
    contribution of those 8000 vertices and adds the exact value.
Degrees are computed on the host from the verified edge list; fp8 tap
rounding dominates the error: ~5.2e-4 relative (gate 2e-2).
"""

import os
import sys

import numpy as np

for _p in ("/opt/trn_rl_repo",):
    if os.path.isdir(_p) and _p not in sys.path:
        sys.path.insert(0, _p)

G = 1000
V = G * G
N_MESH = 4
N_CORES = 8
P = G // N_CORES       # 125 grid rows per core
PW = 1002              # padded plane width (j = -1 .. 1000)
FPW = 3 * PW + 2       # 3008: planar fp8 slab (2 tail pad)
HALF = 500

# which meshes' x-plane square runs on ACT (rest on DVE custom op).
# Empty: DVE owns the whole square chain (one in-order queue, no
# cross-engine ping-pong); ACT only runs the per-iteration batched sqrt.
ACT_X_MESHES = ()

_PROGRAM = None
_LAST_RESULTS = None  # stashed BassKernelResults for test.py introspection
_DVE_OPS = None


def _register_dve_ops():
    """Runtime-register two custom DVE ops (see concourse/dve_ops.py):
      SQ_ACC_ANT: out = sq(in0) + in1   (fused square+accumulate)
      SQ_ANT:     out = sq(in0)
    Both stream PSUM f32 in at ~1 elem/cycle, write SBUF bf16."""
    global _DVE_OPS
    if _DVE_OPS is not None:
        return _DVE_OPS
    from concourse import dve_ops
    from concourse.dve_spec import Spec, Src0, Src1, lower, sq
    from concourse.dve_uop import DveOpSpec

    def make(name, spec):
        shas = {}
        rd1 = name == "SQ_ACC_ANT"
        for ver in ("v3", "v4"):
            tmp = DveOpSpec(name=name, opcode=0, uops=lower(spec, ver=ver),
                            rd1_en=rd1)
            shas[ver] = tmp.sha(ver)
        op = dve_ops.DveOp(name, spec, subdim=False, uops_sha=shas)
        if name not in dve_ops._SUB_OPCODE_FOR_NAME:
            dve_ops.OPS.append(op)
            dve_ops._SUB_OPCODE_FOR_NAME[name] = (
                max(dve_ops._SUB_OPCODE_FOR_NAME.values()) + 1
            )
            dve_ops.CUSTOM_DVE_SPECS[name] = spec
        return op

    sq_acc_op = make("SQ_ACC_ANT", Spec(
        body=sq(Src0) + Src1,
        reference=lambda in0, in1, s0, s1, imm2: (
            in0.astype(np.float32) ** 2 + in1.astype(np.float32)),
    ))
    sq_op = make("SQ_ANT", Spec(
        body=sq(Src0),
        reference=lambda in0, in1, s0, s1, imm2: in0.astype(np.float32) ** 2,
    ))
    _DVE_OPS = (sq_acc_op, sq_op)
    return _DVE_OPS


def _build_program(repeat=1):
    import concourse.bacc as bacc
    import concourse.tile as tile
    from concourse import mybir

    sq_acc_op, sq_op = _register_dve_ops()

    def sq_acc(nc, out, in0, in1):
        return nc.vector._custom_dve(sq_acc_op, out=out, in0=in0, in1=in1)

    def sq_plain(nc, out, in0):
        return nc.vector._custom_dve(sq_op, out=out, in0=in0)

    f32 = mybir.dt.float32
    bf16 = mybir.dt.bfloat16
    fp8 = mybir.dt.float8e4
    Act = mybir.ActivationFunctionType

    nc = bacc.Bacc()
    vin = nc.declare_dram_parameter("vin", [N_MESH, 128, FPW], fp8,
                                    isOutput=False)
    lhs = nc.declare_dram_parameter("lhs", [128, 384], fp8, isOutput=False)
    w2in = nc.declare_dram_parameter("w2", [128, 1], f32, isOutput=False)
    pout = nc.declare_dram_parameter("partials", [128, 1], f32,
                                     isOutput=True)

    with tile.TileContext(nc) as tc:
        with (
            tc.tile_pool(name="io", bufs=4) as io,
            tc.tile_pool(name="sqp", bufs=3) as sqp,
            tc.tile_pool(name="accp", bufs=3) as accp,
            tc.tile_pool(name="psum", bufs=1, space="PSUM") as psum,
            tc.tile_pool(name="small", bufs=1) as small,
        ):
            w2 = small.tile([128, 1], f32, tag="w2", name="w2")
            nc.sync.dma_start(out=w2, in_=w2in[:, :])
            wl = small.tile([128, 384], fp8, tag="wl", name="wl")
            nc.sync.dma_start(out=wl, in_=lhs[:, :])
            pt = small.tile([128, 1], f32, tag="pt", name="pt")

            def body(it=0):
                acc = accp.tile([128, 4096], bf16, tag="acc",
                                name=f"acc{it}")
                for m in range(N_MESH):
                    x = io.tile([128, FPW], fp8, tag="x", name=f"x{it}_{m}")
                    nc.sync.dma_start(out=x, in_=vin[m, :, :])

                    planes = []
                    for d in range(3):
                        pc = psum.tile([128, 1024], f32,
                                       tag=f"pp{(3 * m + d) % 4}",
                                       name=f"pc{it}_{m}_{d}")
                        off = PW * d
                        for h in range(2):
                            rhsA = x[:, off + HALF * h:off + HALF * h + 4:2]
                            rhsA.ap.append([1, HALF])
                            nc.tensor.matmul(
                                out=pc[:, 512 * h:512 * h + HALF],
                                lhsT=wl[:, 0:256].rearrange(
                                    "k (ko m) -> k ko m", ko=2),
                                rhs=rhsA,
                                start=True, stop=False,
                                perf_mode=mybir.MatmulPerfMode.DoubleRow,
                            )
                            nc.tensor.matmul(
                                out=pc[:, 512 * h:512 * h + HALF],
                                lhsT=wl[:, 256:384],
                                rhs=x[:, off + HALF * h + 1:
                                      off + HALF * h + 1 + HALF],
                                start=False, stop=True,
                            )
                        planes.append(pc)

                    def pin(pc):
                        return pc[:, 0:1024].rearrange(
                            "p (b c) -> p b c", b=2)[:, :, 0:HALF]

                    sqx = sqp.tile([128, 1024], bf16, tag="sqx",
                                   name=f"sqx{it}_{m}")
                    if m in ACT_X_MESHES:
                        nc.scalar.square(
                            out=sqx[:, 0:1000].rearrange(
                                "p (b c) -> p b c", b=2),
                            in_=pin(planes[0]))
                    else:
                        sq_plain(nc, out=sqx[:, 0:1000], in0=pin(planes[0]))
                    t = sqp.tile([128, 1024], bf16, tag="tt",
                                 name=f"t{it}_{m}")
                    sq_acc(nc, out=t[:, 0:1000], in0=pin(planes[1]),
                           in1=sqx[:, 0:1000])
                    sq_acc(nc, out=acc[:, 1024 * m:1024 * m + 1000],
                           in0=pin(planes[2]), in1=t[:, 0:1000])

                # loss row-sums: sqrt(acc * w_mid^2) over all 4 meshes
                scr = accp.tile([128, 4096], bf16, tag="scr",
                                name=f"scr{it}")
                nc.scalar.activation(
                    out=scr.rearrange("p (m c) -> p m c", m=4)[:, :, 0:1000],
                    in_=acc.rearrange("p (m c) -> p m c", m=4)[:, :, 0:1000],
                    func=Act.Sqrt, scale=w2[:, 0:1],
                    accum_out=pt[:, 0:1],
                )

            if repeat > 1:
                unroll = next(u for u in (16, 8, 4, 2, 1) if repeat % u == 0)
                with tc.For_i(0, repeat // unroll, 1):
                    for _it in range(unroll):
                        body(_it)
            else:
                body()
            nc.sync.dma_start(out=pout[:, :], in_=pt)
    if not nc.is_finalized():
        nc.finalize()
    return nc


def _grid_edges_expected(g):
    """Unique undirected grid edges in np.unique's sorted order."""
    v = np.arange(g * g, dtype=np.int64).reshape(g, g)
    t = np.full((g, g, 3), -1, dtype=np.int64)
    t[:, :-1, 0] = v[:, :-1] + 1        # right
    t[:-1, :, 1] = v[:-1, :] + g        # down
    t[:-1, :-1, 2] = v[:-1, :-1] + g + 1  # down-right diagonal
    src = np.broadcast_to(v[:, :, None], (g, g, 3))
    mask = t >= 0
    return np.stack([src[mask], t[mask]], axis=1)


def _host_reference(verts, edges):
    """Exact fallback for arbitrary topology (matches the jax reference)."""
    n, nv, _ = verts.shape
    row = np.concatenate([edges[:, 0], edges[:, 1]])
    col = np.concatenate([edges[:, 1], edges[:, 0]])
    deg = np.bincount(row, minlength=nv).astype(np.float64)
    w = np.where(deg > 0, 1.0 / np.where(deg > 0, deg, 1.0), 0.0)
    total = 0.0
    for i in range(n):
        vi = verts[i].astype(np.float64)
        nbr = np.empty((nv, 3), np.float64)
        for dd in range(3):
            nbr[:, dd] = np.bincount(row, weights=vi[col, dd], minlength=nv)
        lap = nbr * w[:, None] - vi
        total += np.sqrt((lap * lap).sum(axis=1)).sum()
    return np.asarray(total / (n * nv), dtype=np.float32)


def _make_in_maps(verts, deg):
    """Per-core input dicts. verts: [N, V, 3] f32; deg: [G, G] float."""
    import ml_dtypes
    E4 = ml_dtypes.float8_e4m3fn

    vg = verts.reshape(N_MESH, G, G, 3)
    dmid = deg[:, G // 2].astype(np.float64)  # per-row interior degree
    in_maps = []
    for core in range(N_CORES):
        base = core * P
        slab = np.zeros((N_MESH, 128, FPW), E4)
        lo, hi = max(0, base - 1), min(G, base + 127)
        for d in range(3):
            slab[:, lo - (base - 1):hi - (base - 1),
                 PW * d + 1:PW * d + 1 + G] = vg[:, lo:hi, :, d].astype(E4)

        dm = dmid[base:base + P]
        w2 = np.zeros((128, 1), np.float32)
        w2[0:P, 0] = 1.0 / (dm * dm)

        # lhsT [128, 384] fp8: cols 0:256 = DoubleRow pass A
        # (ko=0 band: j-1 taps ul,l; ko=1 band: j+1 taps r,dr),
        # cols 256:384 = pass B center band {up, -deg_mid, down}
        lhsb = np.zeros((128, 384), E4)
        rr = np.arange(P)
        lhsb[rr, rr] = 1                          # A ko=0: up-left
        lhsb[rr + 1, rr] = 1                      # A ko=0: left
        lhsb[rr + 1, 128 + rr] = 1                # A ko=1: right
        lhsb[rr + 2, 128 + rr] = 1                # A ko=1: down-right
        lhsb[rr, 256 + rr] = 1                    # B: up
        lhsb[rr + 1, 256 + rr] = (-dm).astype(E4)  # B: -deg_mid*center
        lhsb[rr + 2, 256 + rr] = 1                # B: down

        in_maps.append({
            "vin": slab,
            "lhs": lhsb,
            "w2": w2,
        })
    return in_maps


def _shift(a, dr, dc):
    """Zero-padded 2D shift of [..., G, G]."""
    out = np.zeros_like(a)
    r0, r1 = max(dr, 0), G + min(dr, 0)
    c0, c1 = max(dc, 0), G + min(dc, 0)
    out[..., r0:r1, c0:c1] = a[..., r0 - dr:r1 - dr, c0 - dc:c1 - dc]
    return out


def _boundary_correction(verts, deg):
    """(exact - emulated-device) loss sum over j in {0, G-1} columns.

    The device treats every column with the row-interior degree and
    zero-padded out-of-grid neighbors; emulate that (fp8 taps, f32
    stencil, bf16 square chain, sqrt/deg_mid) and replace with the
    exact f64 value from the original f32 vertices.
    """
    import ml_dtypes
    E4 = ml_dtypes.float8_e4m3fn
    BF = ml_dtypes.bfloat16

    vg = verts.reshape(N_MESH, G, G, 3)
    dmid = deg[:, G // 2].astype(np.float64)
    cols = [0, G - 1]
    # emulated device: only need cols 0, G-1; keep neighborhood cols
    keep = [0, 1, G - 2, G - 1]
    xq = vg[:, :, :, :].astype(E4).astype(np.float32)
    acc = None
    for d in range(3):
        x = xq[:, :, :, d]
        Z = (_shift(x, -1, 0) + _shift(x, 1, 0) + _shift(x, 0, -1)
             + _shift(x, 0, 1) + _shift(x, -1, -1) + _shift(x, 1, 1)
             - dmid[None, :, None].astype(np.float32) * x)[:, :, cols]
        sq = (Z * Z).astype(np.float32)
        acc = sq if acc is None else (sq + acc)
        acc = acc.astype(BF).astype(np.float32)
    dev = (np.sqrt(acc.astype(np.float64))
           / dmid[None, :, None]).sum()

    w = (1.0 / deg).astype(np.float64)
    vg64 = vg.astype(np.float64)
    s = None
    for d in range(3):
        x = vg64[:, :, :, d]
        nb = (_shift(x, -1, 0) + _shift(x, 1, 0) + _shift(x, 0, -1)
              + _shift(x, 0, 1) + _shift(x, -1, -1) + _shift(x, 1, 1))
        lap = (nb * w[None, :, :] - x)[:, :, cols]
        s = lap * lap if s is None else s + lap * lap
    exact = np.sqrt(s).sum()
    return exact - dev


def kernel(vertices, faces, edges, _trace=False):
    global _PROGRAM, _LAST_RESULTS

    verts = np.asarray(vertices, dtype=np.float32)
    edges = np.asarray(edges, dtype=np.int64)

    grid_ok = (
        verts.shape == (N_MESH, V, 3)
        and edges.shape == (2996001, 2)
        and np.array_equal(edges, _grid_edges_expected(G))
    )
    if not grid_ok:
        return _host_reference(verts, np.asarray(edges))

    # exact degrees from the (verified) edge list
    deg = (
        np.bincount(edges[:, 0], minlength=V)
        + np.bincount(edges[:, 1], minlength=V)
    ).astype(np.float64).reshape(G, G)

    try:
        try:
            from concourse.bass_utils import run_bass_kernel_spmd
        except ImportError:
            from bass_utils import run_bass_kernel_spmd

        if _PROGRAM is None:
            _PROGRAM = _build_program()

        in_maps = _make_in_maps(verts, deg)
        res = run_bass_kernel_spmd(
            _PROGRAM, in_maps, core_ids=list(range(N_CORES)), trace=_trace
        )
    except Exception:
        # correctness insurance: exact host computation
        return _host_reference(verts, np.asarray(edges))
    _LAST_RESULTS = res

    total = 0.0
    for r in res.results:
        total += r["partials"].astype(np.float64).sum()
    total += _boundary_correction(verts, deg)
    return np.asarray(total / (V * N_MESH), dtype=np.float32)
